# revision 14
# baseline (speedup 1.0000x reference)
"""Trainium2 Bass kernel for nn_Disentangler (gnn_message_passing).

Math (per timestamp t, derived from the reference):
  - encode LayerNorm over D of x rows; only rows at node_pos are used.
  - scatter to nodes by node_ids, adaptive-avg-pool D->C, segment-mean over
    L=8 groups of 4096 nodes  ==>  comp[l] = A1 @ (sum over selected rows p
    with node_ids//4096==l of LN(x_p)) / 4096.
    LN(x_p) = (x_p - m_p) * r_p * g_enc + b_enc with per-row mean m_p and
    r_p = 1/sqrt(var_p + eps).  So the bucket sums only need
    W_l = sum r_p x_p (a one-hot-weighted matmul) plus S_l = sum r_p m_p =
    reduce_sum(W_l)/D, and counts cnt_l.
  - LN over the L*C=128 comp values (g_fin/b_fin), then per-group LN over
    C=16 (g_dec/b_dec), then upsample C->D by repeat-8.
  - out rows within a group are all identical -> write each group's 4096
    identical rows via a stride-0 broadcast DMA from a [128, 2048] fp16
    replicated tile (4KB packets sourced from 128 partitions in parallel).

Implementation notes (v3, memory-roofline focused):
  - host pre-gathers the P=8192 selected rows per timestamp (node_pos) and
    ships them fp16: read traffic 4.2 MB/core instead of 16.8 MB full-x f32.
  - bucket one-hot is built on-chip from a [128, 64] bucket-id tensor via
    is_equal against an iota, scaled by the per-row r.
  - output y is fp16 on device (host upcasts during unshard): write traffic
    16.8 MB/core instead of 33.6.  fp16 error ~5e-4 << 2e-2 tolerance.
  - all metadata (bid/cnt/LN params/iota/ones) rides in ONE [128, 467] f32
    blob with a single dma_start, so the scalar queue is free to start the
    Square chain as soon as the first x chunk lands.
  - bucket matmul runs reversed: lhsT = x tile (stationary), rhs = sel
    (moving, 8 cols) -> ~200ns/tile instead of ~400ns; accumulates
    W.T [128, 8] in PSUM; one PE transpose recovers W [8, 128] per t.
  - all x-chunk loads are issued up front on the sync queue before the
    y-write DMAs (no head-of-line blocking of loads behind writes).

Sharding: data-parallel over T=16 timestamps across 8 cores (2 per core).
"""

import numpy as np

import concourse.bass as bass
import concourse.bacc as bacc
import concourse.tile as tile
from concourse import mybir
from concourse.bass_utils import run_bass_kernel_spmd
from concourse.masks import make_identity

F32 = mybir.dt.float32
F16 = mybir.dt.float16
AF = mybir.ActivationFunctionType
ALU = mybir.AluOpType
AX = mybir.AxisListType

T, TOK, D, N, L, C = 16, 16384, 128, 32768, 8, 16
P = 8192                    # selected rows per timestamp
NCORES = 8
TLOC = T // NCORES          # timestamps per core
NT = P // 128               # 64 row-tiles per timestamp
CH = 4                      # x chunks per timestamp
JPC = NT // CH              # 16 tiles per chunk
GRP = N // L                # 4096 nodes per group
EPS = 1e-5
POOL_SCALE = 1.0 / ((D // C) * GRP)   # A1 avg (1/8) * segment mean (1/4096)
RW = 2048                   # replicated row-image width (16 copies of D)
BW = 467                    # blob width (f32 columns)

_CACHE = {}


def _build():
    nc = bacc.Bacc("TRN2", debug=False)
    xs = nc.dram_tensor("xs", [TLOC, P, D], F16, kind="ExternalInput")
    blob = nc.dram_tensor("blob", [128, BW], F32, kind="ExternalInput")
    bsel = nc.dram_tensor("bsel", [L, L * 128], F16, kind="ExternalInput")
    y = nc.dram_tensor("y", [TLOC, N, D], F16, kind="ExternalOutput")

    with tile.TileContext(nc) as tc:
        with (
            tc.tile_pool(name="xp", bufs=1) as xp,
            tc.tile_pool(name="sqp", bufs=2) as sqp,
            tc.tile_pool(name="selp", bufs=2) as selp,
            tc.tile_pool(name="stat", bufs=4) as stat,
            tc.tile_pool(name="mid", bufs=2) as mid,
            tc.tile_pool(name="rep", bufs=2) as repp,
            tc.tile_pool(name="repg", bufs=4) as repg,
            tc.tile_pool(name="const", bufs=1) as cst,
            tc.tile_pool(name="pswt", bufs=2, space="PSUM") as pswt,
            tc.tile_pool(name="psw", bufs=2, space="PSUM") as psw,
            tc.tile_pool(name="pst", bufs=2, space="PSUM") as pst,
            tc.tile_pool(name="psb", bufs=2, space="PSUM") as psb,
        ):
            # ---- x chunk loads: issue ALL of them first on the sync queue ----
            xch = []
            for t in range(TLOC):
                xr = xs[t].rearrange("(p j) d -> p j d", p=128)
                for c in range(CH):
                    xc = xp.tile([128, JPC, D], F16, tag=f"x{t}c{c}")
                    nc.sync.dma_start(out=xc[:], in_=xr[:, c * JPC:(c + 1) * JPC, :])
                    xch.append(xc)

            # ---- metadata blob: ONE dma_start on the scalar queue ----
            blob_s = cst.tile([128, BW], F32)
            nc.scalar.dma_start(out=blob_s[:], in_=blob[:])
            bsel_s = cst.tile([L, L * 128], F16)
            nc.scalar.dma_start(out=bsel_s[:], in_=bsel[:])
            bid_t = [blob_s[:, t * NT:(t + 1) * NT] for t in range(TLOC)]
            iot_v = blob_s[:, 128:136]
            geb_v = blob_s[0:L, 136:264]
            beb_v = blob_s[0:L, 264:392]
            gft_v = blob_s[0:L, 392:408]
            bft_v = blob_s[0:L, 408:424]
            gdt_v = blob_s[0:L, 424:440]
            bdt_v = blob_s[0:L, 440:456]
            cnt_t = [blob_s[0:L, 456 + t:457 + t] for t in range(TLOC)]
            on8_v = blob_s[0:L, 458:459]
            on18_v = blob_s[0:1, 459:467]

            eps_s = cst.tile([128, 1], F32); nc.vector.memset(eps_s[:], EPS)
            eps2_s = cst.tile([1, 1], F32); nc.vector.memset(eps2_s[:], EPS / (POOL_SCALE * POOL_SCALE))
            ident = cst.tile([128, 128], F32)
            make_identity(nc, ident[:])

            for t in range(TLOC):
                wt_ps = pswt.tile([128, L], F32)   # accumulates W.T over tiles
                for c in range(CH):
                    xc = xch[t * CH + c]
                    sums = stat.tile([128, JPC], F32, tag="sums")
                    nc.vector.reduce_sum(out=sums[:], in_=xc[:], axis=AX.X)
                    xsq = sqp.tile([128, JPC * D], F16)
                    nc.scalar.activation(out=xsq[:],
                                         in_=xc[:].rearrange("p j d -> p (j d)"),
                                         func=AF.Square)
                    sumsq = stat.tile([128, JPC], F32, tag="sumsq")
                    nc.vector.reduce_sum(out=sumsq[:],
                                         in_=xsq[:].rearrange("p (j d) -> p j d", d=D),
                                         axis=AX.X)
                    s2 = stat.tile([128, JPC], F32, tag="s2")
                    nc.gpsimd.tensor_mul(out=s2[:], in0=sums[:], in1=sums[:])
                    nc.gpsimd.tensor_scalar(out=s2[:], in0=s2[:], scalar1=1.0 / D,
                                            scalar2=None, op0=ALU.mult)
                    nc.gpsimd.tensor_tensor(out=s2[:], in0=sumsq[:], in1=s2[:],
                                            op=ALU.subtract)
                    r16 = stat.tile([128, JPC], F16, tag="r")
                    nc.scalar.activation(out=r16[:], in_=s2[:], func=AF.Abs_reciprocal_sqrt,
                                         bias=eps_s[:], scale=1.0 / D)
                    sel = selp.tile([128, JPC, L], F16)
                    nc.vector.tensor_tensor(
                        out=sel[:],
                        in0=bid_t[t][:, c * JPC:(c + 1) * JPC].rearrange(
                            "p (j o) -> p j o", o=1).to_broadcast([128, JPC, L]),
                        in1=iot_v.rearrange("p (o l) -> p o l", o=1).to_broadcast(
                            [128, JPC, L]),
                        op=ALU.is_equal)
                    nc.vector.tensor_tensor(
                        out=sel[:], in0=sel[:],
                        in1=r16[:].rearrange("p (j o) -> p j o", o=1).to_broadcast(
                            [128, JPC, L]),
                        op=ALU.mult)
                    for jj in range(JPC):
                        j = c * JPC + jj
                        nc.tensor.matmul(wt_ps[:], lhsT=xc[:, jj, :], rhs=sel[:, jj, :],
                                         start=(j == 0), stop=(j == NT - 1))

                # W.T [128, 8] -> W [8, 128] via PE transpose
                wts = mid.tile([128, L], F32, tag="wts")
                nc.vector.tensor_copy(out=wts[:], in_=wt_ps[:])
                ps_w = psw.tile([L, D], F32)
                nc.tensor.transpose(ps_w[:], in_=wts[:], identity=ident[:])

                # ---- per-timestamp tail (all tiny, f32) ----
                S = mid.tile([L, 1], F32, tag="S")
                nc.vector.reduce_sum(out=S[:], in_=ps_w[:], axis=AX.X)
                nc.vector.tensor_scalar(out=S[:], in0=S[:], scalar1=1.0 / D,
                                        scalar2=None, op0=ALU.mult)
                t1 = mid.tile([L, D], F32, tag="t1")
                nc.vector.tensor_scalar(out=t1[:], in0=ps_w[:], scalar1=S[:],
                                        scalar2=None, op0=ALU.subtract)
                nc.vector.tensor_mul(out=t1[:], in0=t1[:], in1=geb_v)
                cb = mid.tile([L, D], F32, tag="cb")
                nc.vector.tensor_scalar_mul(out=cb[:], in0=beb_v, scalar1=cnt_t[t])
                nc.vector.tensor_add(out=t1[:], in0=t1[:], in1=cb[:])

                cp = mid.tile([L, C], F32, tag="cp")
                nc.vector.reduce_sum(out=cp[:], in_=t1[:].rearrange("l (c g) -> l c g", g=D // C),
                                     axis=AX.X)

                # LN over all L*C values: stats via ones-matmul partition sum
                sq = mid.tile([L, C], F32, tag="sq")
                nc.vector.tensor_mul(out=sq[:], in0=cp[:], in1=cp[:])
                ps2 = pst.tile([1, 2 * C], F32, tag="tail")
                nc.tensor.matmul(ps2[:, :C], lhsT=on8_v, rhs=cp[:], start=True, stop=True)
                nc.tensor.matmul(ps2[:, C:], lhsT=on8_v, rhs=sq[:], start=True, stop=True)
                su = mid.tile([1, 2], F32, tag="su")
                nc.vector.reduce_sum(out=su[:], in_=ps2[:].rearrange("p (a c) -> p a c", a=2),
                                     axis=AX.X)
                mst = mid.tile([1, 2], F32, tag="mst")
                nc.vector.tensor_scalar(out=mst[:], in0=su[:], scalar1=1.0 / (L * C),
                                        scalar2=None, op0=ALU.mult)  # [mean, meansq]
                msq = mid.tile([1, 1], F32, tag="msq")
                nc.vector.tensor_mul(out=msq[:], in0=mst[:, 0:1], in1=mst[:, 0:1])
                var = mid.tile([1, 1], F32, tag="var")
                nc.vector.tensor_tensor(out=var[:], in0=mst[:, 1:2], in1=msq[:],
                                        op=ALU.subtract)
                nc.scalar.activation(out=mst[:, 1:2], in_=var[:], func=AF.Abs_reciprocal_sqrt,
                                     bias=eps2_s[:1, :], scale=1.0)
                psb2 = pst.tile([L, 2], F32, tag="tail")
                nc.tensor.matmul(psb2[:], lhsT=on18_v, rhs=mst[:], start=True, stop=True)
                bsb = mid.tile([L, 2], F32, tag="bsb")
                nc.vector.tensor_copy(out=bsb[:], in_=psb2[:])

                cl = mid.tile([L, C], F32, tag="cl")
                nc.vector.tensor_scalar(out=cl[:], in0=cp[:], scalar1=bsb[:, 0:1],
                                        scalar2=bsb[:, 1:2],
                                        op0=ALU.subtract, op1=ALU.mult)
                nc.vector.tensor_mul(out=cl[:], in0=cl[:], in1=gft_v)
                nc.vector.tensor_add(out=cl[:], in0=cl[:], in1=bft_v)

                # decode LN over C per group
                st2 = mid.tile([L, nc.vector.BN_STATS_DIM], F32, tag="st2")
                nc.vector.bn_stats(out=st2[:], in_=cl[:])
                mv2 = mid.tile([L, 2], F32, tag="mv2")
                nc.vector.bn_aggr(out=mv2[:], in_=st2[:])
                r2 = mid.tile([L, 1], F32, tag="r2")
                nc.scalar.activation(out=r2[:], in_=mv2[:, 1:2], func=AF.Abs_reciprocal_sqrt,
                                     bias=eps_s[:L, :], scale=1.0)
                dn = mid.tile([L, C], F32, tag="dn")
                nc.vector.tensor_scalar(out=dn[:], in0=cl[:], scalar1=mv2[:, 0:1],
                                        scalar2=r2[:],
                                        op0=ALU.subtract, op1=ALU.mult)
                nc.vector.tensor_mul(out=dn[:], in0=dn[:], in1=gdt_v)
                nc.vector.tensor_add(out=dn[:], in0=dn[:], in1=bdt_v)

                # upsample C -> D (repeat 8), fp16
                rw = repp.tile([L, D], F16)
                nc.vector.tensor_copy(
                    out=rw[:].rearrange("l (c k) -> l c k", k=D // C),
                    in_=dn[:].rearrange("l (c u) -> l c u", u=1).to_broadcast(
                        [L, C, D // C]))

                # broadcast each group's row-image to all 128 partitions with
                # a narrow PE ones-column matmul ([128, D] only), widen 16x
                # during the PSUM->SBUF fp16 cast copy (stride-0 input), then
                # write the group's 4096 identical rows with a 2x stride-0
                # re-read: 4KB packets from 128 partitions.
                nrep = GRP * D // (128 * RW)
                for gl in range(L):
                    pb = psb.tile([128, D], F32)
                    nc.tensor.matmul(pb[:],
                                     lhsT=bsel_s[:, gl * 128:(gl + 1) * 128],
                                     rhs=rw[:], start=True, stop=True)
                    rep = repg.tile([128, RW], F16)
                    rep_out = rep[:].rearrange("p (r d) -> p r d", d=D)
                    rep_in = pb[:].rearrange("p (o d) -> p o d", o=1).to_broadcast(
                        [128, RW // D, D])
                    if gl % 2 == 0:
                        nc.scalar.copy(out=rep_out, in_=rep_in)
                    else:
                        nc.vector.tensor_copy(out=rep_out, in_=rep_in)
                    out_ap = y[t, gl * GRP:(gl + 1) * GRP, :].rearrange(
                        "(p a f) d -> p a (f d)", p=128, a=nrep)
                    in_ap = rep[:].rearrange("p (o f) -> p o f", o=1).to_broadcast(
                        [128, nrep, RW])
                    nc.sync.dma_start(out=out_ap, in_=in_ap)

    nc.compile()
    return nc


def _get_nc():
    if "nc" not in _CACHE:
        _CACHE["nc"] = _build()
    return _CACHE["nc"]


def _host_prep(x, g_enc, b_enc, g_fin, b_fin, g_dec, b_dec, node_pos, node_ids):
    """Build per-core input maps: gather selected rows (fp16) + metadata blob."""
    x = np.asarray(x)
    node_pos = np.asarray(node_pos)
    buckets = (np.asarray(node_ids) // GRP).astype(np.int64)          # [T, P]

    base = np.zeros((128, BW), np.float32)
    base[:, 128:136] = np.arange(L, dtype=np.float32)
    base[0:L, 136:264] = np.asarray(g_enc, np.float32)
    base[0:L, 264:392] = np.asarray(b_enc, np.float32)
    base[0:L, 392:408] = np.asarray(g_fin, np.float32).reshape(L, C)
    base[0:L, 408:424] = np.asarray(b_fin, np.float32).reshape(L, C)
    base[0:L, 424:440] = np.asarray(g_dec, np.float32)
    base[0:L, 440:456] = np.asarray(b_dec, np.float32)
    base[0:L, 458] = 1.0
    base[0, 459:467] = 1.0
    bsel = np.zeros((L, L * 128), np.float16)
    for l in range(L):
        bsel[l, l * 128:(l + 1) * 128] = 1.0

    in_maps = []
    for core in range(NCORES):
        xs = np.empty((TLOC, P, D), np.float16)
        blob = base.copy()
        for i, t in enumerate(range(core * TLOC, (core + 1) * TLOC)):
            xs[i] = x[t][node_pos[t]]
            blob[:, i * NT:(i + 1) * NT] = buckets[t].astype(np.float32).reshape(128, NT)
            blob[0:L, 456 + i] = np.bincount(buckets[t], minlength=L).astype(np.float32)
        in_maps.append({"xs": xs, "blob": blob, "bsel": bsel})
    return in_maps


def kernel(**inputs):
    in_maps = _host_prep(
        inputs["x"], inputs["g_enc"], inputs["b_enc"], inputs["g_fin"], inputs["b_fin"],
        inputs["g_dec"], inputs["b_dec"], inputs["node_pos"], inputs["node_ids"])
    nc = _get_nc()
    res = run_bass_kernel_spmd(nc, in_maps, core_ids=list(range(NCORES)))
    out = np.concatenate([r["y"] for r in res.results], axis=0)
    return out.astype(np.float32)
